# revision 17
# baseline (speedup 1.0000x reference)
"""Trainium2 Bass kernel for a 3D attention block.

Reference computation (per batch b):
    xf = x[b].reshape(C, N)                       # C=256, N=4096
    q  = Wq @ xf + bq                             # [32, N]
    k  = Wk @ xf + bk                             # [32, N]
    v  = Wv @ xf + bv                             # [256, N]
    P  = softmax(q.T @ k, axis=-1)                # [N(m), N(n)]
    out[c, m] = sum_n v[c, n] * P[m, n]
    result = gamma * out + x[b]

Sharding: 8 cores = 2 batches x 4 chunks of 1024 query rows (m).
Each core gets the full xf[b] (for k, v) plus its own 1024-column chunk
(for q and the residual), and writes out[:, chunk] of shape [256, 1024].

On-device layout (per core) is transpose-free:
    S^T[n, m] = k^T q   (n on partitions)  -> exp on ACT -> P^T in SBUF
    out[c, m] = sum over n-tiles of vT[n-tile, c].T @ P^T[n-tile, m]
    rowsum[m] = ones[n].T @ P^T  (PSUM [1, m])
Softmax max-subtraction is skipped (|S| <= ~30, exp stays in fp32 range).

Precision: x and the weights are shipped as fp16 (q/k/v magnitudes are
only a few units, well inside fp16 range); q/k and the S^T matmuls run
in fp16, exp produces P in bf16 (exp(S) can reach ~4e12, beyond fp16
range), and the PV + rowsum matmuls run in bf16. PSUM accumulation is
always fp32, softmax normalization and the residual add are fp32.
Validated against the fp32 reference: absmax ~5e-3 on an output scale
of ~5.3 (~9e-4 scale-relative).

ATTN_KERNEL_REPEATS=<R> emits the body R times in one NEFF (timing via
slope; outputs are idempotent). ATTN_KERNEL_TRACE=1 captures an NTFF
profile via run_bass_kernel_spmd(trace=True).
"""

import os

import numpy as np

import concourse.bass as bass
import concourse.mybir as mybir
import concourse.tile as tile
from concourse import bacc
from concourse.bass_utils import run_bass_kernel_spmd

F32 = mybir.dt.float32
F16 = mybir.dt.float16
BF16 = mybir.dt.bfloat16

C = 256
C8 = 32
N = 4096  # 16*16*16 voxels
MCHUNK = 1024  # query rows per core
NT = N // 128  # 32 key tiles
NCORES = 8

# info stashed by the last kernel() call (for test harnesses)
LAST_RESULTS = None


def _emit_body(nc, tc, io, rep):
    xin, xch, xres, wqt, wkt, wvt, bq, bk, bv, gamma, out = io
    r = f"_{rep}"
    with (
        tc.tile_pool(name="big" + r, bufs=1) as big,
        tc.tile_pool(name="ptp" + r, bufs=5) as ptp,
        tc.tile_pool(name="epi" + r, bufs=2) as epi,
        tc.tile_pool(name="pacc" + r, bufs=1, space="PSUM") as pacc,
        tc.tile_pool(name="pst" + r, bufs=2, space="PSUM") as pst,
    ):
        # ---- inputs on one HWDGE queue, ordered so the projection
        # matmuls start as early as possible
        xc = [big.tile([128, MCHUNK], F16, name=f"xc{h}" + r) for h in range(2)]
        wq_t = [big.tile([128, C8], F16, name=f"wq{h}" + r) for h in range(2)]
        wk_t = [big.tile([128, C8], F16, name=f"wk{h}" + r) for h in range(2)]
        wv_t = [big.tile([128, C], F16, name=f"wv{h}" + r) for h in range(2)]
        xf = [big.tile([128, N], F16, name=f"xf{h}" + r) for h in range(2)]
        for h in range(2):
            nc.sync.dma_start(xc[h][:], xch[h * 128 : (h + 1) * 128, :])
        for h in range(2):
            rows = slice(h * 128, (h + 1) * 128)
            nc.sync.dma_start(wq_t[h][:], wqt[rows, :])
        for h in range(2):
            rows = slice(h * 128, (h + 1) * 128)
            nc.sync.dma_start(wk_t[h][:], wkt[rows, :])
        for h in range(2):
            nc.sync.dma_start(xf[h][:, 0:1024], xin[h * 128 : (h + 1) * 128, 0:1024])
        for h in range(2):
            rows = slice(h * 128, (h + 1) * 128)
            nc.sync.dma_start(wv_t[h][:], wvt[rows, :])
        bq_t = big.tile([C8, 1], F32, name="bq_t" + r)
        bk_t = big.tile([C8, 1], F32, name="bk_t" + r)
        nc.sync.dma_start(bq_t[:], bq[:])
        nc.sync.dma_start(bk_t[:], bk[:])
        for h in range(2):
            nc.sync.dma_start(
                xf[h][:, 1024:2048], xin[h * 128 : (h + 1) * 128, 1024:2048]
            )
        bv_b = big.tile([128, C], F32, name="bv_b" + r)
        nc.sync.dma_start(
            bv_b[:],
            bass.AP(tensor=bv, offset=0, ap=[[0, 128], [1, C]]),
        )
        gamma_t = big.tile([1, 1], F32, name="gamma_t" + r)
        nc.sync.dma_start(gamma_t[:], gamma[:])
        for ch in range(2, 4):
            sl = slice(ch * 1024, (ch + 1) * 1024)
            for h in range(2):
                nc.sync.dma_start(xf[h][:, sl], xin[h * 128 : (h + 1) * 128, sl])
        xr = [big.tile([128, MCHUNK], F32, name=f"xr{h}" + r) for h in range(2)]
        for h in range(2):
            nc.sync.dma_start(xr[h][:], xres[h * 128 : (h + 1) * 128, :])
        ones_t = big.tile([128, 1], BF16, name="ones_t" + r)
        nc.vector.memset(ones_t[:], 1.0)
        ones_row = big.tile([1, 128], F32, name="ones_row" + r)
        nc.vector.memset(ones_row[:], 1.0)


        # ---- projections (ordered by xf chunk arrival) ----
        q_sb = big.tile([C8, MCHUNK], F16, name="q_sb" + r)
        for mh in range(2):
            sl = slice(mh * 512, (mh + 1) * 512)
            q_ps = pst.tile([128, 512], F32, tag="st", name=f"q_ps{mh}" + r)
            nc.tensor.matmul(
                q_ps[:C8, :], wq_t[0][:], xc[0][:, sl], start=True, stop=False
            )
            nc.tensor.matmul(
                q_ps[:C8, :], wq_t[1][:], xc[1][:, sl], start=False, stop=True
            )
            nc.vector.tensor_scalar_add(q_sb[:, sl], q_ps[:C8, :], bq_t[:])

        k_sb = big.tile([C8, N], F16, name="k_sb" + r)
        vt_sb = big.tile([128, NT, C], BF16, name="vt_sb" + r)

        def emit_k(ch):
            sl = slice(ch * 512, (ch + 1) * 512)
            k_ps = pst.tile([128, 512], F32, tag="st", name=f"k_ps{ch}" + r)
            nc.tensor.matmul(
                k_ps[:C8, :], wk_t[0][:], xf[0][:, sl], start=True, stop=False
            )
            nc.tensor.matmul(
                k_ps[:C8, :], wk_t[1][:], xf[1][:, sl], start=False, stop=True
            )
            nc.vector.tensor_scalar_add(k_sb[:, sl], k_ps[:C8, :], bk_t[:])

        def emit_vt(nt):
            sl = slice(nt * 128, (nt + 1) * 128)
            v_ps = pst.tile([128, 512], F32, tag="st", name=f"v_ps{nt}" + r)
            nc.tensor.matmul(
                v_ps[:, :C], xf[0][:, sl], wv_t[0][:], start=True, stop=False
            )
            nc.tensor.matmul(
                v_ps[:, :C], xf[1][:, sl], wv_t[1][:], start=False, stop=True
            )
            nc.vector.tensor_add(vt_sb[:, nt, :], v_ps[:, :C], bv_b[:])

        for grp in range(4):
            emit_k(2 * grp)
            emit_k(2 * grp + 1)
            for nt in range(8 * grp, 8 * grp + 8):
                emit_vt(nt)

        # ---- main attention loop ----
        # acc[h] accumulates out[c-half, m]; rs accumulates rowsums [1, m]
        acc = [pacc.tile([128, MCHUNK], F32, name=f"acc{h}" + r) for h in range(2)]
        rs_ps = pacc.tile([1, MCHUNK], F32, name="rs_ps" + r)

        pts = [None] * NT

        def emit_pv(i):
            first, last = i == 0, i == NT - 1
            pt = pts[i]
            if not last:  # the last tile's rowsum is emitted early, inline
                for mh in range(2):
                    msl = slice(mh * 512, (mh + 1) * 512)
                    nc.tensor.matmul(
                        rs_ps[:, msl], ones_t[:], pt[:, msl], start=first, stop=False
                    )
            for h in range(2):
                vsl = vt_sb[:, i, h * 128 : (h + 1) * 128]
                for mh in range(2):
                    msl = slice(mh * 512, (mh + 1) * 512)
                    nc.tensor.matmul(
                        acc[h][:, msl], vsl, pt[:, msl], start=first, stop=last
                    )

        for nt in range(NT):
            ksl = k_sb[:, nt * 128 : (nt + 1) * 128]
            st = [
                pst.tile([128, 512], F32, tag="st", name=f"st{nt}_{i}" + r)
                for i in range(2)
            ]
            pt = ptp.tile([128, MCHUNK], BF16, tag="pt", name=f"pt{nt}" + r)
            for mh in range(2):
                msl = slice(mh * 512, (mh + 1) * 512)
                nc.tensor.matmul(
                    st[mh][:], ksl, q_sb[:, msl], start=True, stop=True
                )
                nc.scalar.activation(
                    pt[:, msl], st[mh][:], mybir.ActivationFunctionType.Exp
                )
            pts[nt] = pt
            if nt == NT - 1:
                # rowsum of the last tile first: the epilogue's
                # normalization chain depends only on rs_ps
                for mh in range(2):
                    msl = slice(mh * 512, (mh + 1) * 512)
                    nc.tensor.matmul(
                        rs_ps[:, msl], ones_t[:], pt[:, msl],
                        start=False, stop=True,
                    )
            if nt >= 1:
                emit_pv(nt - 1)
        emit_pv(NT - 1)

        # ---- epilogue: scale by gamma/rowsum, add residual, store ----
        # rs_sc = rowsum / gamma (ACT), reciprocal on DVE, then broadcast
        # across partitions with a K=1 matmul (PE is idle by now), all
        # pipelined in two m-halves.
        ginv = epi.tile([1, 1], F32, name="ginv" + r)
        nc.vector.reciprocal(ginv[:], gamma_t[:])
        rs_sc = epi.tile([1, MCHUNK], F32, name="rs_sc" + r)
        rs_rec = epi.tile([1, MCHUNK], F32, name="rs_rec" + r)
        grecip_b = big.tile([128, MCHUNK], F32, name="gr_b" + r)
        res = [
            epi.tile([128, MCHUNK], F32, tag=f"res{h}", name=f"res{h}" + r)
            for h in range(2)
        ]
        for mh in range(2):
            msl = slice(mh * 512, (mh + 1) * 512)
            nc.scalar.activation(
                rs_sc[:, msl], rs_ps[:, msl],
                mybir.ActivationFunctionType.Copy, scale=ginv[:],
            )
            nc.vector.reciprocal_approx_fast(rs_rec[:, msl], rs_sc[:, msl])
            gr_ps = pst.tile([128, 512], F32, tag="st", name=f"gr_ps{mh}" + r)
            nc.tensor.matmul(
                gr_ps[:], ones_row[:], rs_rec[:, msl], start=True, stop=True
            )
            nc.scalar.copy(grecip_b[:, msl], gr_ps[:])
            for h in range(2):
                nc.vector.tensor_mul(res[h][:, msl], acc[h][:, msl], grecip_b[:, msl])
                nc.vector.tensor_add(res[h][:, msl], res[h][:, msl], xr[h][:, msl])
                nc.sync.dma_start(
                    out[h * 128 : (h + 1) * 128, msl], res[h][:, msl]
                )


def _build(repeats=1):
    nc = bacc.Bacc("TRN2", target_bir_lowering=False, debug=False, num_devices=NCORES)

    xin = nc.dram_tensor("xin", [C, N], F16, kind="ExternalInput")
    xch = nc.dram_tensor("xch", [C, MCHUNK], F16, kind="ExternalInput")
    xres = nc.dram_tensor("xres", [C, MCHUNK], F32, kind="ExternalInput")
    wqt = nc.dram_tensor("wqt", [C, C8], F16, kind="ExternalInput")
    wkt = nc.dram_tensor("wkt", [C, C8], F16, kind="ExternalInput")
    wvt = nc.dram_tensor("wvt", [C, C], F16, kind="ExternalInput")
    bq = nc.dram_tensor("bq", [C8, 1], F32, kind="ExternalInput")
    bk = nc.dram_tensor("bk", [C8, 1], F32, kind="ExternalInput")
    bv = nc.dram_tensor("bv", [1, C], F32, kind="ExternalInput")
    gamma = nc.dram_tensor("gamma", [1, 1], F32, kind="ExternalInput")
    out = nc.dram_tensor("out", [C, MCHUNK], F32, kind="ExternalOutput")
    io = (xin, xch, xres, wqt, wkt, wvt, bq, bk, bv, gamma, out)

    with tile.TileContext(nc) as tc:
        for rep in range(repeats):
            _emit_body(nc, tc, io, rep)

    nc.compile()
    return nc


_NC_CACHE = {}


def _get_nc(repeats=1):
    if repeats not in _NC_CACHE:
        _NC_CACHE[repeats] = _build(repeats)
    return _NC_CACHE[repeats]


def _in_maps(x, Wq, bq, Wk, bk, Wv, bv, gamma):
    xflat = x.reshape(2, C, N)
    xflat16 = xflat.astype(np.float16)
    wqt = np.ascontiguousarray(Wq.T.astype(np.float16))  # [C, C8]
    wkt = np.ascontiguousarray(Wk.T.astype(np.float16))
    wvt = np.ascontiguousarray(Wv.T.astype(np.float16))  # [C_in, C_out]
    bq2 = np.ascontiguousarray(bq.reshape(C8, 1))
    bk2 = np.ascontiguousarray(bk.reshape(C8, 1))
    bv2 = np.ascontiguousarray(bv.reshape(1, C))
    g2 = np.ascontiguousarray(gamma.reshape(1, 1))

    maps = []
    for core in range(NCORES):
        b, j = core // 4, core % 4
        maps.append(
            {
                "xin": np.ascontiguousarray(xflat16[b]),
                "xch": np.ascontiguousarray(
                    xflat16[b][:, j * MCHUNK : (j + 1) * MCHUNK]
                ),
                "xres": np.ascontiguousarray(
                    xflat[b][:, j * MCHUNK : (j + 1) * MCHUNK]
                ),
                "wqt": wqt,
                "wkt": wkt,
                "wvt": wvt,
                "bq": bq2,
                "bk": bk2,
                "bv": bv2,
                "gamma": g2,
            }
        )
    return maps


def kernel(x, Wq, bq, Wk, bk, Wv, bv, gamma):
    global LAST_RESULTS
    x = np.ascontiguousarray(np.asarray(x, dtype=np.float32))
    args = [np.asarray(a, dtype=np.float32) for a in (Wq, bq, Wk, bk, Wv, bv, gamma)]

    B, Cc, D, H, W = x.shape
    assert (B, Cc, D * H * W) == (2, C, N), x.shape

    repeats = int(os.environ.get("ATTN_KERNEL_REPEATS", "1"))
    nc = _get_nc(repeats)
    maps = _in_maps(x, *args)
    kwargs = {}
    if int(os.environ.get("ATTN_KERNEL_TRACE", "0")):
        kwargs = dict(
            trace=True,
            trace_cores=[0],
            tmpdir=os.environ.get("ATTN_KERNEL_TRACE_DIR"),
        )
    res = run_bass_kernel_spmd(nc, maps, core_ids=list(range(NCORES)), **kwargs)
    LAST_RESULTS = res

    outf = np.empty((B, C, N), dtype=np.float32)
    for core in range(NCORES):
        b, j = core // 4, core % 4
        outf[b][:, j * MCHUNK : (j + 1) * MCHUNK] = res.results[core]["out"]
    return outf.reshape(B, Cc, D, H, W)
